# revision 27
# baseline (speedup 1.0000x reference)
"""CARAFE kernel for 8 TRN2 NeuronCores (Bass/Tile, SPMD).

Math (see reference):
  k0   = w_comp @ x + b_comp                 (64, 32, 32)      1x1 conv
  kc   = w_ker (*) k0 + b_ker                (102400, 32, 32)  3x3 conv, pad 1
  k    = softmax(kc.reshape(4, 25600, H, W), axis=1)
  ksum = k.sum(axis=1)                       (4, 32, 32)       == S/S (==1+eps)
  out  = (x[:, :, None] * ksum[:, None]).reshape(1, 256, 64, 64)

Sharding: core c = (g, h) with g = c//2 (softmax/scale group), h = c%2
(pixel half: image rows 16h..16h+16).  Each core computes its group's
FULL 25600 conv channels for its OWN 512 pixels, so the softmax group
sum S is core-local -- no collective at all (the baseline's pairwise
AllReduce cost ~29us of pure tail latency).

Device layout choices:
  * The 3x3 conv is evaluated in fp8e4 with DoubleRow perf mode: each
    matmul contracts 256 im2col rows (2 fp8 weights/cell), streaming
    N=512 channels.  Contraction split: ktile A = taps 0-3, ktile B =
    taps 4-7 (DoubleRow, 256 rows each), tap 8 as a 64-row fp8 tail.
  * Weights are scaled x16 on the host before fp8 quantization (their
    0.05 sigma sits in e4m3's subnormal range); the Exp eviction's
    activation scale of 1/16 undoes it exactly.  Softmax sums are
    divided by themselves (ksum == 1 in exact arithmetic), so conv
    precision does not reach the output.
  * im2col without materialization: the compressed image (18 rows incl
    halo) lives in a 23-row x 32-col zero-framed fp8 strip; each tap is
    a flat-shifted SBUF->SBUF DMA copy of it (shift (dh-1)*32+(dw-1)).
    Row-crossing leaks of the dw!=1 shifts are zeroed with tiny column
    memsets, so the conv is exact.  A 128-pixel matmul window (4 image
    rows) is then a single contiguous 128B slice -- flat 3D DoubleRow
    lhsT APs.  Weight-tile prefetch is issued AFTER the copy triggers
    so the blocked copy semaphores never stall the weight stream.
  * The stationary operand is the image window, shared by consecutive
    matmuls (channel-tile pairs), keeping LDWEIGHTS off the critical
    path; psum tiles span 2 banks ([128, 1024]) holding a channel-tile
    pair, halving ScalarE eviction instruction count.
  * tap-8 tails for a channel-tile pair are packed as two concurrent
    row-tiled K=64 matmuls (array rows 0-63 / 64-127), costing ~one
    matmul slot instead of two.  b_ker / b_comp are all zeros for this
    problem's setup_inputs; a separate exact variant (K=65 ones row +
    halo mask) is built lazily if nonzero biases are ever passed.
  * Exp eviction on ScalarE produces per-pixel partial softmax sums via
    accum_out (spart is mt-major so each m-tile's final reduce has a
    clean contiguous dependency range); the finals (reduce, S*(1/S),
    x multiply, store) are pipelined per m-tile so output write-back
    overlaps the last channel pairs.
  * All DRAM tensors are host-pre-transposed to partition-major layouts
    so every DMA moves multi-KB contiguous per-partition segments; the
    512KB output store is split across 4 queue triggers.
"""

import numpy as np

import concourse.bass as bass
import concourse.mybir as mybir
import concourse.tile as tile
from concourse import bacc
from concourse.bass_utils import run_bass_kernel_spmd

F32 = mybir.dt.float32
BF16 = mybir.dt.bfloat16
FP8 = mybir.dt.float8e4
AF = mybir.ActivationFunctionType
DR = mybir.MatmulPerfMode.DoubleRow

# Problem constants
C, H, W = 256, 32, 32
CH = 64                    # compressed channels
SC = 2                     # upsample scale
OC_TOTAL = 102400
NCORES = 8
GCH = OC_TOTAL // 4        # 25600 channels per softmax group (= per core)
NT = GCH // 512            # 50 channel tiles of 512
NPAIR = NT // 2            # 25 channel-tile pairs
PIX = 512                  # pixels per core (16 image rows)
MT = PIX // 128            # 4 pixel tiles of 128 (4 image rows each)
HLOC = 18                  # local k0 rows incl 1-row halo each side
NLOC = HLOC * W            # 576 compress-conv pixels
FROWS = 23                 # zero-framed strip rows
FRAME = FROWS * W          # 736 bytes per image copy (div by 16)
WSCALE = 16.0              # host weight scale, undone by Exp's 1/16

# frame row f holds k0 local row f-3 (local rows -1..16 at f=2..19)
EV0 = 2 * W                # eviction start: flat offset of frame row 2
# matmul window for m-tile mt: local rows 4mt..4mt+3 -> frame rows
# 4mt+3..4mt+6 -> flat [32*(4mt+3), +128)
def WOFF(mt):
    return W * (4 * mt + 3)

# tap t = (dh, dw) = (t//3, t%3); copy shift = (dh-1)*32 + (dw-1)
# DoubleRow ktile A: (i, phalf) -> tap [[0, 1], [2, 3]]; B: [[4, 5], [6, 7]]
A_TAPS = [[0, 1], [2, 3]]
B_TAPS = [[4, 5], [6, 7]]


def build(with_bias=False):
    nc = bacc.Bacc("TRN2", target_bir_lowering=False, debug=False,
                   num_devices=NCORES)

    xf = nc.dram_tensor("xf", [128, 2, NLOC], BF16, kind="ExternalInput")
    xt = nc.dram_tensor("xt", [128, MT, C], F32, kind="ExternalInput")
    wc = nc.dram_tensor("wc", [128, 2, CH], BF16, kind="ExternalInput")
    bc = nc.dram_tensor("bc", [CH, 1], F32, kind="ExternalInput")
    wk = nc.dram_tensor("wk", [NT, 128, 2, 2, 512], FP8, kind="ExternalInput")
    if with_bias:
        wkt = nc.dram_tensor("wkt", [NT, 65, 512], FP8, kind="ExternalInput")
        hm = nc.dram_tensor("hm", [CH, NLOC], FP8, kind="ExternalInput")
    else:
        wkt = nc.dram_tensor("wkt", [NPAIR, 128, 512], FP8, kind="ExternalInput")
    out = nc.dram_tensor("out", [128, MT, C], F32, kind="ExternalOutput")
    sdbg = nc.dram_tensor("sdbg", [128, MT], F32, kind="ExternalOutput")

    with tile.TileContext(nc) as tc:
        with (
            tc.tile_pool(name="const", bufs=1) as const,
            tc.tile_pool(name="wpool", bufs=8) as wpool,
            tc.tile_pool(name="tpool", bufs=4) as tpool,
            tc.tile_pool(name="ppool", bufs=4, space="PSUM") as ppool,
            tc.tile_pool(name="epool", bufs=3) as epool,
        ):
            def load_wt(n):
                wt = wpool.tile([128, 2, 2, 512], FP8, tag="wt", name=f"wt_{n}")
                nc.sync.dma_start(wt[:], wk.ap()[n])
                return wt

            def load_tail(pair):
                shape = [65 if with_bias else 128, 512]
                tt = tpool.tile(shape, FP8, tag="tt", name=f"tt_{pair}")
                nc.sync.dma_start(tt[:], wkt.ap()[pair])
                return tt

            # ---- input staging (compress-conv operands only; the weight
            # prefetch is issued after the copy triggers below so a blocked
            # copy semaphore never stalls the weight stream) ----
            x_sb = const.tile([128, 2, NLOC], BF16)
            nc.sync.dma_start(x_sb[:], xf.ap())
            wc_sb = const.tile([128, 2, CH], BF16)
            nc.sync.dma_start(wc_sb[:], wc.ap())
            bc_sb = const.tile([CH, 1], F32)
            nc.sync.dma_start(bc_sb[:], bc.ap())

            # preload the activation table while input DMAs are in flight
            # (a lazy ACT_TABLE_LOAD costs 1.3us on the compress critical path)
            warm = const.tile([1, 16], F32)
            nc.vector.memset(warm[:], 0.0)
            nc.scalar.activation(warm[:], warm[:], AF.Exp)

            # image strips: U1 = ktile A (taps 0-3), U2 = ktile B (taps 4-7),
            # V8 = tap 8 duplicated on both partition halves.
            U1 = const.tile([128, 2, FRAME], FP8)
            U2 = const.tile([128, 2, FRAME], FP8)
            V8 = const.tile([128, FRAME], FP8)
            # base strip = U2[0:64, 0] (tap 4, shift 0): zero its frame edges
            nc.vector.memset(U2[0:64, 0, 0:EV0], 0.0)
            nc.vector.memset(U2[0:64, 0, EV0 + NLOC:FRAME], 0.0)

            # ---- compress conv: k0 = w_comp @ x + b_comp, evict as fp8;
            # second half evicted by DVE so the two halves run in parallel ----
            base = U2[0:64, 0, :]
            cps = []
            for nh in range(2):
                ps = ppool.tile([128, 1024], F32, tag="ps", name=f"cps_{nh}")
                for kt in range(2):
                    nc.tensor.matmul(
                        ps[0:CH, 0:NLOC // 2],
                        lhsT=wc_sb[:, kt, :],
                        rhs=x_sb[:, kt, nh * (NLOC // 2):(nh + 1) * (NLOC // 2)],
                        start=(kt == 0), stop=(kt == 1),
                    )
                cps.append(ps)
            nc.scalar.activation(base[:, EV0:EV0 + NLOC // 2],
                                 cps[0][0:CH, 0:NLOC // 2],
                                 AF.Identity, bias=bc_sb[:])
            nc.vector.tensor_scalar_add(base[:, EV0 + NLOC // 2:EV0 + NLOC],
                                        cps[1][0:CH, 0:NLOC // 2], bc_sb[:])
            if with_bias:
                # halo rows hold b_comp instead of the conv's zero padding;
                # mask them (b_comp == 0 on the fast path makes them exact)
                hm_sb = const.tile([CH, NLOC], FP8)
                nc.gpsimd.dma_start(hm_sb[:], hm.ap())
                nc.vector.tensor_mul(base[:, EV0:EV0 + NLOC],
                                     base[:, EV0:EV0 + NLOC], hm_sb[:])

            # ---- 9 flat shifted copies of the base strip (SBUF->SBUF DMA;
            # engine tensor_copy measured ~4x slower on fp8).  U1 (ktile A)
            # first: it gates the first matmuls. ----
            CPY0, CPY1 = 2 * W, 21 * W          # dst copy extent [64, 672)
            def tapcopy(dst, t, eng):
                s = (t // 3 - 1) * W + (t % 3 - 1)
                eng.dma_start(dst[:, CPY0:CPY1], base[:, CPY0 + s:CPY1 + s])
            tapcopy(U1[0:64, 0, :], 0, nc.sync)
            tapcopy(U1[64:128, 0, :], 1, nc.gpsimd)
            tapcopy(U1[0:64, 1, :], 2, nc.scalar)
            tapcopy(U1[64:128, 1, :], 3, nc.sync)
            tapcopy(U2[0:64, 1, :], 6, nc.gpsimd)
            tapcopy(U2[64:128, 0, :], 5, nc.scalar)
            tapcopy(U2[64:128, 1, :], 7, nc.sync)
            tapcopy(V8[0:64, :], 8, nc.gpsimd)
            if with_bias:
                nc.vector.memset(V8[64:65, :], 1.0)    # bias ones row
            else:
                tapcopy(V8[64:128, :], 8, nc.scalar)

            # weight prefetch, after the copy triggers
            wts = {0: load_wt(0), 1: load_wt(1)}
            tts = {0: load_tail(0)} if not with_bias else {0: load_tail(0),
                                                           1: load_tail(1)}

            # zero the row-crossing leak columns (dw=0 -> col 0, dw=2 -> col
            # 31) over the window rows 3..18, all on the otherwise-idle DVE
            def colfix(strip, col):
                ap = strip.rearrange("p (r c) -> p r c", c=W)
                nc.vector.memset(ap[:, 3:19, col:col + 1], 0.0)
            colfix(U1[0:64, 0, :], 0)        # tap 0
            colfix(U1[0:64, 1, :], 31)       # tap 2
            colfix(U1[64:128, 1, :], 0)      # tap 3
            colfix(U2[64:128, 0, :], 31)     # tap 5
            colfix(U2[0:64, 1, :], 0)        # tap 6
            colfix(V8[0:64, :], 31)          # tap 8
            if not with_bias:
                colfix(V8[64:128, :], 31)    # tap 8 dup

            # x^T for the output stage: not needed until the end, loaded on
            # the Pool SWDGE queue after the copies
            xt_sb = const.tile([128, MT, C], F32)
            nc.gpsimd.dma_start(xt_sb[:], xt.ap())

            # ---- big conv + exp + per-pixel partial sums ----
            # spart is mt-major: column mt*NPAIR + pair
            spart = const.tile([128, MT * NPAIR], F32)
            for pair in range(NPAIR):
                n0, n1 = 2 * pair, 2 * pair + 1
                wt0 = wts.pop(n0) if n0 in wts else load_wt(n0)
                wt1 = wts.pop(n1) if n1 in wts else load_wt(n1)
                if with_bias:
                    tt0 = tts.pop(n0) if n0 in tts else load_tail(n0)
                    tt1 = tts.pop(n1) if n1 in tts else load_tail(n1)
                else:
                    tt = tts.pop(pair) if pair in tts else load_tail(pair)
                for mt in range(MT):
                    w0, w1 = WOFF(mt), WOFF(mt) + 128
                    lhsA = U1[:, :, w0:w1]
                    lhsB = U2[:, :, w0:w1]
                    pt = ppool.tile([128, 1024], F32, tag="ps",
                                    name=f"pt_{pair}_{mt}")
                    nc.tensor.matmul(pt[:, 0:512], lhsT=lhsA, rhs=wt0[:, 0],
                                     start=True, stop=False, perf_mode=DR)
                    nc.tensor.matmul(pt[:, 512:1024], lhsT=lhsA, rhs=wt1[:, 0],
                                     start=True, stop=False, perf_mode=DR)
                    nc.tensor.matmul(pt[:, 0:512], lhsT=lhsB, rhs=wt0[:, 1],
                                     start=False, stop=False, perf_mode=DR)
                    nc.tensor.matmul(pt[:, 512:1024], lhsT=lhsB, rhs=wt1[:, 1],
                                     start=False, stop=False, perf_mode=DR)
                    if with_bias:
                        # ones row lives at V8[64] / tt[64]; K=65, serial
                        nc.tensor.matmul(pt[:, 0:512],
                                         lhsT=V8[0:65, w0:w1], rhs=tt0[:],
                                         start=False, stop=True)
                        nc.tensor.matmul(pt[:, 512:1024],
                                         lhsT=V8[0:65, w0:w1], rhs=tt1[:],
                                         start=False, stop=True)
                    else:
                        # packed K=64 tails: concurrent row-tiled matmuls
                        nc.tensor.matmul(pt[:, 0:512],
                                         lhsT=V8[0:64, w0:w1], rhs=tt[0:64, :],
                                         start=False, stop=True)
                        nc.tensor.matmul(pt[:, 512:1024],
                                         lhsT=V8[64:128, w0:w1], rhs=tt[64:128, :],
                                         start=False, stop=True)
                    et = epool.tile([128, 1024], BF16, tag="et")
                    idx = mt * NPAIR + pair
                    nc.scalar.activation(et[:], pt[:], AF.Exp,
                                         scale=1.0 / WSCALE,
                                         accum_out=spart[:, idx:idx + 1])

            # ---- per-mt finals: S -> ksum = S/S -> out = x^T * ksum ----
            # pipelined per m-tile so the store overlaps the last pairs
            S = const.tile([128, MT], F32)
            rec = const.tile([128, MT], F32)
            ks = const.tile([128, MT], F32)
            ot = const.tile([128, MT, C], F32)
            st_eng = (nc.sync, nc.scalar, nc.gpsimd, nc.sync)
            for mt in range(MT):
                nc.vector.tensor_reduce(
                    S[:, mt:mt + 1], spart[:, mt * NPAIR:(mt + 1) * NPAIR],
                    axis=mybir.AxisListType.X, op=mybir.AluOpType.add,
                )
                nc.vector.reciprocal(rec[:, mt:mt + 1], S[:, mt:mt + 1])
                nc.vector.tensor_mul(ks[:, mt:mt + 1], S[:, mt:mt + 1],
                                     rec[:, mt:mt + 1])
                nc.vector.tensor_scalar_mul(
                    ot[:, mt, :], xt_sb[:, mt, :], ks[:, mt:mt + 1],
                )
                st_eng[mt].dma_start(out.ap()[:, mt], ot[:, mt, :])
            nc.scalar.dma_start(sdbg.ap(), S[:])

    nc.compile()
    return nc


_NC = {}


def _get_nc(with_bias=False):
    if with_bias not in _NC:
        _NC[with_bias] = build(with_bias)
    return _NC[with_bias]


def _pmajor(a, p=128):
    """(k*p, n...) row-major -> (p, k, n...) partition-major."""
    k = a.shape[0] // p
    return np.ascontiguousarray(a.reshape(k, p, *a.shape[1:]).transpose(
        1, 0, *range(2, a.ndim + 1)))


def prep_inputs(x, w_comp, b_comp, w_ker, b_ker):
    import ml_dtypes
    E4 = ml_dtypes.float8_e4m3
    x = np.asarray(x, dtype=np.float32).reshape(C, H, W)
    w_comp = np.asarray(w_comp, dtype=np.float32)
    b_comp = np.asarray(b_comp, dtype=np.float32)
    w_ker = np.asarray(w_ker, dtype=np.float32)
    b_ker = np.asarray(b_ker, dtype=np.float32)
    with_bias = bool(np.any(b_ker)) or bool(np.any(b_comp))

    xp = np.zeros((C, H + 2, W), np.float32)
    xp[:, 1:H + 1] = x
    wcT = _pmajor(np.ascontiguousarray(
        w_comp.reshape(CH, C).T).astype(ml_dtypes.bfloat16))
    bcr = np.ascontiguousarray(b_comp.reshape(CH, 1), dtype=np.float32)

    # weights: x16 scale, fp8e4, grouped [nt, p=hi*64+ci, kt, i, n]
    w9 = (w_ker.reshape(OC_TOTAL, CH, 9) * WSCALE).astype(E4)
    bk16 = (b_ker * WSCALE).astype(E4)

    in_maps = []
    for core in range(NCORES):
        g, h = core // 2, core % 2
        xfc = _pmajor(np.ascontiguousarray(
            xp[:, 16 * h:16 * h + HLOC].reshape(C, NLOC)
        ).astype(ml_dtypes.bfloat16))
        xtc = _pmajor(np.ascontiguousarray(
            x.reshape(C, H * W)[:, PIX * h:PIX * (h + 1)].T))
        a = w9[GCH * g:GCH * (g + 1)].reshape(NT, 512, CH, 9)
        wkc = np.empty((NT, 128, 2, 2, 512), E4)
        for kt, taps in enumerate((A_TAPS, B_TAPS)):
            for i in range(2):
                for hi in range(2):
                    wkc[:, 64 * hi:64 * (hi + 1), kt, i, :] = (
                        a[:, :, :, taps[i][hi]].transpose(0, 2, 1))
        t8 = a[:, :, :, 8].transpose(0, 2, 1)          # (NT, 64, 512)
        if with_bias:
            wktc = np.empty((NT, 65, 512), E4)
            wktc[:, 0:64] = t8
            wktc[:, 64] = bk16[GCH * g:GCH * (g + 1)].reshape(NT, 512)
        else:
            wktc = np.ascontiguousarray(t8.reshape(NPAIR, 128, 512))
        im = {
            "xf": xfc,
            "xt": xtc,
            "wc": wcT,
            "bc": bcr,
            "wk": np.ascontiguousarray(wkc),
            "wkt": wktc,
        }
        if with_bias:
            hmv = np.ones((CH, HLOC, W), np.float32)
            hmv[:, 0 if h == 0 else HLOC - 1] = 0.0
            im["hm"] = hmv.reshape(CH, NLOC).astype(E4)
        in_maps.append(im)
    return in_maps, with_bias


def assemble(results):
    full = np.empty((C, 4, H, W), dtype=np.float32)
    for core in range(NCORES):
        g, h = core // 2, core % 2
        blk = results[core]["out"]                     # (128, 4, 256)
        pix = blk.transpose(1, 0, 2).reshape(PIX, C)   # (512, 256)
        full[:, g, 16 * h:16 * (h + 1), :] = pix.T.reshape(C, 16, W)
    return full.reshape(1, C, SC * H, SC * W)


def run(in_maps, with_bias=False, trace=False, **kw):
    nc = _get_nc(with_bias)
    return run_bass_kernel_spmd(nc, in_maps, list(range(NCORES)), trace=trace, **kw)


def kernel(x, w_comp, b_comp, w_ker, b_ker):
    in_maps, with_bias = prep_inputs(x, w_comp, b_comp, w_ker, b_ker)
    res = run(in_maps, with_bias)
    return assemble(res.results)
